# revision 1
# baseline (speedup 1.0000x reference)
# Trainium2 Bass kernel for nn_Krop_81544249082422 (4-layer Qwen3-style
# transformer, alternating full / sliding-window attention).
#
# Sharding: 8 cores = (batch 4) x (seq-half 2). Each core owns 512 tokens of
# one batch element, feature-major ([feature, token]) through the whole stack.
# Cross-core traffic: pairwise K/V AllGather per full-attn layer; 12-token
# halo exchange per sliding layer.
import sys

for p in ("/opt/trn_rl_repo", "/opt/pypackages"):
    if p not in sys.path:
        sys.path.insert(0, p)

import numpy as np
import ml_dtypes

import concourse.bass as bass
import concourse.bacc as bacc
import concourse.mybir as mybir
import concourse.tile as tile
from concourse import bass_utils

F32 = mybir.dt.float32
BF16 = mybir.dt.bfloat16
AF = mybir.ActivationFunctionType

L, D, H, HK, HD, FF = 4, 1024, 16, 8, 64, 3072
WIN = 12
THETA = 1e6
EPS = 1e-6
B, S = 4, 1024
NCORES = 8
T = 512            # tokens per core
NC_D = D // 128    # 8 feature chunks
NC_T = T // 128    # 4 local token chunks
QKV_OUT = H * HD + 2 * HK * HD   # 2048
VAUG = HK * (HD + 1)             # 520: 8 kv heads x (64 + ones col)
KOFF = 32                        # ext-k column offset (local token 0 -> col 32)
KEXT = T + 2 * KOFF              # 576
KE = NC_T * 128 * 24             # 12288 (k edge block in halo exchange)
VE = 24 * VAUG                   # 12480
HALO = KE + VE                   # 24768
PAIRS = [[0, 1], [2, 3], [4, 5], [6, 7]]
# q-head slot layout: chunk c rows [0:64) = QPERM[0][c], rows [64:128) = QPERM[1][c].
# Chosen so each q head's GQA kv head sits at the same partition parity
# (matmul requires equal base partitions for lhsT and rhs).
QPERM = [[0, 1, 4, 5, 8, 9, 12, 13], [2, 3, 6, 7, 10, 11, 14, 15]]


def _build_program(n_cores=NCORES):
    nc = bacc.Bacc("TRN2", target_bir_lowering=False, debug=False,
                   num_devices=n_cores)

    def din(name, shape, dt=BF16):
        return nc.dram_tensor(name, shape, dt, kind="ExternalInput").ap()

    X = din("x", [D, T], F32)
    WQKV = din("wqkv", [L, D, QKV_OUT])
    WO = din("wo", [L, D, D])
    WGU = din("wgu", [L, D, 2 * FF])
    WDN = din("wdn", [L, FF, D])
    QBC = din("qbc", [L, 2, 128], F32)
    KBC = din("kbc", [L, 2, 128], F32)
    COSB = din("cosb", [128, T], F32)
    SINB = din("sinb", [128, T], F32)
    ROPEP = din("ropeP", [128, 128])
    BLKSUM = din("blksum", [128, 2], F32)
    ONES128 = din("ones128", [128, 1], F32)
    ONES1X = din("ones1x", [1, 128], F32)
    ID128 = din("id128", [128, 128])
    SM0 = din("sm0", [64, 128])
    SM1 = din("sm1", [128, 128])
    SM2 = din("sm2", [32, 128])
    SMEL = din("smel", [12, 128])
    SMER = din("smer", [12, 128])
    NW = din("nw", [128, NC_D], F32)
    OUT = nc.dram_tensor("out", [D, T], F32, kind="ExternalOutput").ap()

    with tile.TileContext(nc) as tc:
        cst = tc.alloc_tile_pool(name="cst", bufs=1)
        st = tc.alloc_tile_pool(name="st", bufs=1)
        p_nrm = tc.alloc_tile_pool(name="p_nrm", bufs=8)
        p_sq = tc.alloc_tile_pool(name="p_sq", bufs=2)
        p_sm = tc.alloc_tile_pool(name="p_sm", bufs=4)
        p_bcs = tc.alloc_tile_pool(name="p_bcs", bufs=2)
        p_qn = tc.alloc_tile_pool(name="p_qn", bufs=2)
        p_t12 = tc.alloc_tile_pool(name="p_t12", bufs=3)
        p_qf = tc.alloc_tile_pool(name="p_qf", bufs=8)
        p_kloc = tc.alloc_tile_pool(name="p_kloc", bufs=4)
        p_kbig = tc.alloc_tile_pool(name="p_kbig", bufs=4)
        p_vsb = tc.alloc_tile_pool(name="p_vsb", bufs=2)
        p_vtok = tc.alloc_tile_pool(name="p_vtok", bufs=8)
        p_es = tc.alloc_tile_pool(name="p_es", bufs=3)
        p_prod = tc.alloc_tile_pool(name="p_prod", bufs=24)
        p_osb = tc.alloc_tile_pool(name="p_osb", bufs=2)
        p_wqkv = tc.alloc_tile_pool(name="p_wqkv", bufs=8)
        p_wo = tc.alloc_tile_pool(name="p_wo", bufs=8)
        p_wg = tc.alloc_tile_pool(name="p_wg", bufs=9)
        p_wu = tc.alloc_tile_pool(name="p_wu", bufs=9)
        p_wdn = tc.alloc_tile_pool(name="p_wdn", bufs=3)
        psum = tc.alloc_tile_pool(name="psum", bufs=8, space="PSUM")
        dram = tc.alloc_tile_pool(name="dram", bufs=2, space="DRAM")

        def ptile(shape, dt=F32, name="ps"):
            return psum.tile(shape, dt, tag="acc", name=name)

        # ---- load constants ----
        cosb = cst.tile([128, T], F32, name="cosb")
        sinb = cst.tile([128, T], F32, name="sinb")
        ropeP = cst.tile([128, 128], BF16, name="ropeP")
        blksum = cst.tile([128, 2], F32, name="blksum")
        ones128 = cst.tile([128, 1], F32, name="ones128")
        ones1x = cst.tile([1, 128], F32, name="ones1x")
        id128 = cst.tile([128, 128], BF16, name="id128")
        sm0 = cst.tile([128, 128], BF16, name="sm0")
        sm1 = cst.tile([128, 128], BF16, name="sm1")
        sm2 = cst.tile([32, 128], BF16, name="sm2")
        smel = cst.tile([12, 128], BF16, name="smel")
        smer = cst.tile([12, 128], BF16, name="smer")
        nw = cst.tile([128, NC_D], F32, name="nw")
        qbc = cst.tile([2, L * 128], F32, name="qbc")
        kbc = cst.tile([2, L * 128], F32, name="kbc")
        eps2 = cst.tile([2, 1], F32, name="eps2")
        for t_, s_ in ((cosb, COSB), (sinb, SINB), (ropeP, ROPEP),
                       (blksum, BLKSUM), (ones128, ONES128), (ones1x, ONES1X),
                       (id128, ID128), (sm1, SM1), (sm2, SM2),
                       (smel, SMEL), (smer, SMER), (nw, NW)):
            nc.sync.dma_start(out=t_, in_=s_)
        nc.sync.dma_start(out=sm0[64:128, :], in_=SM0)
        for li in range(L):
            nc.sync.dma_start(out=qbc[:, li * 128:(li + 1) * 128], in_=QBC[li])
            nc.sync.dma_start(out=kbc[:, li * 128:(li + 1) * 128], in_=KBC[li])
        nc.vector.memset(eps2, EPS)

        # ---- residual stream ----
        h = []
        for i in range(NC_D):
            hi = st.tile([128, T], F32, name=f"h{i}")
            nc.sync.dma_start(out=hi, in_=X[i * 128:(i + 1) * 128, :])
            h.append(hi)

        def rmsnorm_to(inv_n, out_dt, wmul=None):
            """Compute per-token rstd of h and return list of normed tiles."""
            ss = ptile([1, T], name="ss")
            for i in range(NC_D):
                sq = p_sq.tile([128, T], F32, tag="sq", name="sq")
                nc.scalar.square(sq, h[i])
                nc.tensor.matmul(ss, ones128, sq, start=(i == 0),
                                 stop=(i == NC_D - 1))
            srt = p_sm.tile([1, T], F32, tag="sm", name="srt")
            nc.scalar.activation(srt, ss, AF.Sqrt, bias=eps2[0:1], scale=inv_n)
            rs = p_sm.tile([1, T], F32, tag="sm", name="rs")
            nc.vector.reciprocal(rs, srt)
            bc = ptile([128, T], name="bc")
            nc.tensor.matmul(bc, ones1x, rs, start=True, stop=True)
            outs = []
            for i in range(NC_D):
                o = p_nrm.tile([128, T], out_dt, tag="nrm", name=f"n{i}")
                nc.vector.tensor_mul(o, h[i], bc)
                if wmul is not None:
                    nc.vector.tensor_scalar_mul(o, o, wmul[:, i:i + 1])
                outs.append(o)
            return outs

        def qk_path(li, ps, bcw, out_ap):
            """q/k head-rmsnorm + rope on one [128,T] psum chunk -> out_ap(bf16)."""
            sq = p_sq.tile([128, T], F32, tag="sq", name="qsq")
            nc.scalar.square(sq, ps)
            ss2 = ptile([2, T], name="ss2")
            nc.tensor.matmul(ss2, blksum, sq, start=True, stop=True)
            srt2 = p_sm.tile([2, T], F32, tag="sm", name="srt2")
            nc.scalar.activation(srt2, ss2, AF.Sqrt, bias=eps2, scale=1.0 / HD)
            rs2 = p_sm.tile([2, T], F32, tag="sm", name="rs2")
            nc.vector.reciprocal(rs2, srt2)
            bcq = ptile([128, T], name="bcq")
            nc.tensor.matmul(bcq, bcw[:, li * 128:(li + 1) * 128], rs2,
                             start=True, stop=True)
            bcs = p_bcs.tile([128, T], F32, tag="bcs", name="bcs")
            nc.vector.tensor_copy(bcs, bcq)
            qn = p_qn.tile([128, T], BF16, tag="qn", name="qn")
            nc.vector.tensor_mul(qn, ps, bcs)
            pp = ptile([128, T], name="pp")
            nc.tensor.matmul(pp, ropeP, qn, start=True, stop=True)
            t1 = p_t12.tile([128, T], F32, tag="t12", name="t1")
            nc.vector.tensor_mul(t1, qn, cosb)
            t2 = p_t12.tile([128, T], F32, tag="t12", name="t2")
            nc.vector.tensor_mul(t2, pp, sinb)
            nc.vector.tensor_add(out_ap, t1, t2)

        for li in range(L):
            sliding = (li % 2 == 1)
            n = rmsnorm_to(1.0 / D, BF16)

            # ---- QKV projection + q/k norm/rope + v transpose ----
            wq_sb = []
            for i in range(NC_D):
                w = p_wqkv.tile([128, QKV_OUT], BF16, tag="wqkv", name="wqkv_sb")
                nc.sync.dma_start(out=w, in_=WQKV[li, i * 128:(i + 1) * 128, :])
                wq_sb.append(w)

            qf = []
            kdst = []   # full: kloc tiles [128,T]; sliding: ext_k tiles [128,KEXT]
            if sliding:
                for c in range(NC_T):
                    ek = p_kbig.tile([128, KEXT], BF16, tag="kbig", name=f"extk{c}")
                    kdst.append(ek)
            vdst = []   # local token-major v (+ones cols): [4][128, VAUG]
            for tci in range(NC_T):
                vt = p_vtok.tile([128, VAUG], BF16, tag="vtok", name=f"vt{tci}")
                nc.vector.memset(vt, 1.0)
                vdst.append(vt)
            vf12 = vl12 = vhL = vhR = None
            if sliding:
                vf12 = p_vsb.tile([12, VAUG], BF16, tag="vedge", bufs=8,
                                  name="vf12")
                vl12 = p_vsb.tile([12, VAUG], BF16, tag="vedge", bufs=8,
                                  name="vl12")
                vhL = p_vsb.tile([12, VAUG], BF16, tag="vedge", bufs=8,
                                 name="vhL")
                vhR = p_vsb.tile([12, VAUG], BF16, tag="vedge", bufs=8,
                                 name="vhR")
                for t_ in (vf12, vl12, vhL, vhR):
                    nc.vector.memset(t_, 1.0)

            for j in range(QKV_OUT // 128):
                ps = ptile([128, T], name="qkv_ps")
                for i in range(NC_D):
                    nc.tensor.matmul(ps, wq_sb[i][:, j * 128:(j + 1) * 128],
                                     n[i], start=(i == 0), stop=(i == NC_D - 1))
                if j < 8:
                    q = p_qf.tile([128, T], BF16, tag="qf", name=f"qf{j}")
                    qk_path(li, ps, qbc, q)
                    qf.append(q)
                elif j < 12:
                    c = j - 8
                    if sliding:
                        qk_path(li, ps, kbc, kdst[c][:, KOFF:KOFF + T])
                    else:
                        kl = p_kloc.tile([128, T], BF16, tag="kloc",
                                         name=f"kloc{c}")
                        qk_path(li, ps, kbc, kl)
                        kdst.append(kl)
                else:
                    c = j - 12
                    vsb = p_vsb.tile([128, T], BF16, tag="vsb", name="vsb")
                    nc.vector.tensor_copy(vsb, ps)
                    # token-aligned transposes -> v_tok[tc]
                    for tci in range(NC_T):
                        tr = psum.tile([128, 128], BF16, tag="acc", name="tr")
                        nc.tensor.transpose(
                            tr, vsb[:, tci * 128:(tci + 1) * 128], id128)
                        for hh in range(2):
                            kv = 2 * c + hh
                            nc.vector.tensor_copy(
                                vdst[tci][:, kv * (HD + 1):kv * (HD + 1) + HD],
                                tr[:, hh * HD:(hh + 1) * HD])
                    if sliding:
                        # edge staging: own first/last 12 token rows of v
                        for (stage, a) in ((vf12, 0), (vl12, T - 12)):
                            tre = psum.tile([128, 128], BF16, tag="acc",
                                            name="tre")
                            nc.tensor.transpose(tre[0:12, :], vsb[:, a:a + 12],
                                                id128)
                            for hh in range(2):
                                kv = 2 * c + hh
                                nc.vector.tensor_copy(
                                    stage[:, kv * (HD + 1):kv * (HD + 1) + HD],
                                    tre[0:12, hh * HD:(hh + 1) * HD])

            # ---- K/V exchange ----
            if not sliding:
                cc_in = dram.tile([NC_T, 128, T + VAUG], BF16, tag="ccin",
                                  name="cc_in")
                cc_out = dram.tile([2 * NC_T, 128, T + VAUG], BF16, tag="ccout",
                                   name="cc_out")
                for c in range(NC_T):
                    nc.sync.dma_start(out=cc_in[c, :, 0:T], in_=kdst[c])
                    nc.sync.dma_start(out=cc_in[c, :, T:T + VAUG], in_=vdst[c])
                nc.gpsimd.collective_compute(
                    "AllGather", mybir.AluOpType.bypass, replica_groups=PAIRS,
                    ins=[cc_in.opt()], outs=[cc_out.opt()])
                k_full = []
                for i in range(NC_T):
                    kfl = p_kbig.tile([128, S], BF16, tag="kbig", name=f"kfull{i}")
                    nc.sync.dma_start(out=kfl[:, 0:T], in_=cc_out[i, :, 0:T])
                    nc.sync.dma_start(out=kfl[:, T:S],
                                      in_=cc_out[NC_T + i, :, 0:T])
                    k_full.append(kfl)
                v_aug = []
                for tci in range(2 * NC_T):
                    va = p_vtok.tile([128, VAUG], BF16, tag="vtok",
                                     name=f"vaug{tci}")
                    nc.sync.dma_start(out=va, in_=cc_out[tci, :, T:T + VAUG])
                    v_aug.append(va)
            else:
                cc_in = dram.tile([HALO], BF16, tag="ccin", name="cc_in_s")
                cc_out = dram.tile([2 * HALO], BF16, tag="ccout",
                                   name="cc_out_s")
                kv_view = cc_in[0:KE].rearrange("(c p w) -> c p w", c=NC_T, p=128)
                vv_view = cc_in[KE:HALO].rearrange("(p f) -> p f", p=24)
                for c in range(NC_T):
                    nc.sync.dma_start(out=kv_view[c, :, 0:12],
                                      in_=kdst[c][:, KOFF:KOFF + 12])
                    nc.sync.dma_start(out=kv_view[c, :, 12:24],
                                      in_=kdst[c][:, KOFF + T - 12:KOFF + T])
                nc.sync.dma_start(out=vv_view[0:12, :], in_=vf12)
                nc.sync.dma_start(out=vv_view[12:24, :], in_=vl12)
                nc.gpsimd.collective_compute(
                    "AllGather", mybir.AluOpType.bypass, replica_groups=PAIRS,
                    ins=[cc_in.opt()], outs=[cc_out.opt()])
                e0k = cc_out[0:KE].rearrange("(c p w) -> c p w", c=NC_T, p=128)
                e1k = cc_out[HALO:HALO + KE].rearrange("(c p w) -> c p w",
                                                       c=NC_T, p=128)
                e0v = cc_out[KE:HALO].rearrange("(p f) -> p f", p=24)
                e1v = cc_out[HALO + KE:2 * HALO].rearrange("(p f) -> p f", p=24)
                for c in range(NC_T):
                    nc.sync.dma_start(out=kdst[c][:, KOFF - 12:KOFF],
                                      in_=e0k[c, :, 12:24])
                    nc.sync.dma_start(out=kdst[c][:, KOFF + T:KOFF + T + 12],
                                      in_=e1k[c, :, 0:12])
                nc.sync.dma_start(out=vhL, in_=e0v[12:24, :])
                nc.sync.dma_start(out=vhR, in_=e1v[0:12, :])

            # ---- attention ----
            ao = []
            for i in range(NC_D):
                a = p_nrm.tile([128, T], BF16, tag="nrm", name=f"ao{i}")
                ao.append(a)
            for sl in range(H):
                qc, p = sl // 2, sl % 2
                kv = QPERM[p][qc] // 2
                fc, ro = qc // 2, p * HD
                qo = p * HD
                if not sliding:
                    ctx = ptile([HD + 1, T], name="ctx")
                    for kc in range(S // 128):
                        sT = ptile([128, T], name="sT")
                        nc.tensor.matmul(
                            sT, k_full[fc][ro:ro + HD, kc * 128:(kc + 1) * 128],
                            qf[qc][qo:qo + HD, :], start=True, stop=True)
                        es = p_es.tile([128, T], BF16, tag="es", name="es")
                        nc.scalar.activation(es, sT, AF.Exp)
                        nc.tensor.matmul(
                            ctx, v_aug[kc][:, kv * (HD + 1):(kv + 1) * (HD + 1)],
                            es, start=(kc == 0), stop=(kc == S // 128 - 1))
                    dn = p_sm.tile([1, T], F32, tag="sm", name="dn")
                    nc.vector.reciprocal(dn, ctx[HD:HD + 1, :])
                    bcd = ptile([HD, T], name="bcd")
                    nc.tensor.matmul(bcd, ones1x[0:1, 0:HD], dn,
                                     start=True, stop=True)
                    bcds = p_bcs.tile([128, T], F32, tag="bcs", name="bcds")
                    nc.vector.tensor_copy(bcds[0:HD, :], bcd)
                    nc.vector.tensor_mul(ao[qc][qo:qo + HD, :], ctx[0:HD, :],
                                         bcds[0:HD, :])
                else:
                    vcol = slice(kv * (HD + 1), (kv + 1) * (HD + 1))
                    for ci in range(NC_T):
                        # window pieces (k cols in ext coords, 32-aligned v rows):
                        #  p0: tokens [128ci-32,128ci)   / edge [-12,0) at ci=0
                        #  p1: tokens [128ci, 128ci+128)
                        #  p2: tokens [128ci+128, +32)   / edge [512,524) at ci=3
                        qs = qf[qc][qo:qo + HD, ci * 128:(ci + 1) * 128]
                        pieces = []
                        if ci == 0:
                            pieces.append((12, 0, KOFF - 12,
                                           vhL[0:12, vcol], smel))
                        else:
                            pieces.append(
                                (64, 64, KOFF + ci * 128 - 64,
                                 vdst[ci - 1][64:128, vcol], sm0[64:128, :]))
                        pieces.append(
                            (128, 0, KOFF + ci * 128, vdst[ci][:, vcol], sm1))
                        if ci == NC_T - 1:
                            pieces.append((12, 0, KOFF + T,
                                           vhR[0:12, vcol], smer))
                        else:
                            pieces.append(
                                (32, 0, KOFF + (ci + 1) * 128,
                                 vdst[ci + 1][0:32, vcol], sm2))
                        ctx = ptile([HD + 1, 128], name="ctxs")
                        for pi, (rows, pb, kcol, vap, mask) in enumerate(pieces):
                            sp = ptile([pb + rows, 128], name="sp")
                            nc.tensor.matmul(
                                sp[pb:pb + rows, :],
                                kdst[fc][ro:ro + HD, kcol:kcol + rows],
                                qs, start=True, stop=True)
                            ep = p_es.tile([pb + rows, 128], BF16, tag="es",
                                           name="ep")
                            nc.scalar.activation(ep[pb:pb + rows, :],
                                                 sp[pb:pb + rows, :], AF.Exp)
                            nc.vector.tensor_mul(ep[pb:pb + rows, :],
                                                 ep[pb:pb + rows, :], mask)
                            nc.tensor.matmul(ctx, vap, ep[pb:pb + rows, :],
                                             start=(pi == 0),
                                             stop=(pi == len(pieces) - 1))
                        dn = p_sm.tile([1, 128], F32, tag="sm", name="dns")
                        nc.vector.reciprocal(dn, ctx[HD:HD + 1, :])
                        bcd = ptile([HD, 128], name="bcds_ps")
                        nc.tensor.matmul(bcd, ones1x[0:1, 0:HD], dn,
                                         start=True, stop=True)
                        bcds = p_bcs.tile([128, T], F32, tag="bcs", name="bcdss")
                        nc.vector.tensor_copy(bcds[0:HD, 0:128], bcd)
                        nc.vector.tensor_mul(
                            ao[qc][qo:qo + HD, ci * 128:(ci + 1) * 128],
                            ctx[0:HD, :], bcds[0:HD, 0:128])

            # ---- output projection + residual ----
            wo_sb = []
            for i in range(NC_D):
                w = p_wo.tile([128, D], BF16, tag="wo", name="wo_sb")
                nc.sync.dma_start(out=w, in_=WO[li, i * 128:(i + 1) * 128, :])
                wo_sb.append(w)
            for j in range(NC_D):
                ps = ptile([128, T], name="wo_ps")
                for i in range(NC_D):
                    nc.tensor.matmul(ps, wo_sb[i][:, j * 128:(j + 1) * 128],
                                     ao[i], start=(i == 0), stop=(i == NC_D - 1))
                nc.vector.tensor_add(h[j], h[j], ps)

            # ---- MLP ----
            n2 = rmsnorm_to(1.0 / D, BF16)
            prod = []
            for g in range(6):
                wg_sb, wu_sb = [], []
                for i in range(NC_D):
                    wg_ = p_wg.tile([128, T], BF16, tag="wg", name="wg_sb")
                    nc.sync.dma_start(
                        out=wg_,
                        in_=WGU[li, i * 128:(i + 1) * 128, g * T:(g + 1) * T])
                    wg_sb.append(wg_)
                    wu_ = p_wu.tile([128, T], BF16, tag="wu", name="wu_sb")
                    nc.sync.dma_start(
                        out=wu_,
                        in_=WGU[li, i * 128:(i + 1) * 128,
                                FF + g * T:FF + (g + 1) * T])
                    wu_sb.append(wu_)
                for fl in range(4):
                    gps = ptile([128, T], name="gps")
                    for i in range(NC_D):
                        nc.tensor.matmul(gps, wg_sb[i][:, fl * 128:(fl + 1) * 128],
                                         n2[i], start=(i == 0),
                                         stop=(i == NC_D - 1))
                    ups = ptile([128, T], name="ups")
                    for i in range(NC_D):
                        nc.tensor.matmul(ups, wu_sb[i][:, fl * 128:(fl + 1) * 128],
                                         n2[i], start=(i == 0),
                                         stop=(i == NC_D - 1))
                    gsb = p_sq.tile([128, T], BF16, tag="sq", name="gsb")
                    nc.scalar.activation(gsb, gps, AF.Silu)
                    pr = p_prod.tile([128, T], BF16, tag="prod",
                                     name=f"prod{g * 4 + fl}")
                    nc.vector.tensor_mul(pr, gsb, ups)
                    prod.append(pr)
            dps = [ptile([128, T], name=f"dps{j}") for j in range(NC_D)]
            for i in range(FF // 128):
                wd = p_wdn.tile([128, D], BF16, tag="wdn", name="wd_sb")
                nc.sync.dma_start(out=wd, in_=WDN[li, i * 128:(i + 1) * 128, :])
                for j in range(NC_D):
                    nc.tensor.matmul(dps[j], wd[:, j * 128:(j + 1) * 128],
                                     prod[i], start=(i == 0),
                                     stop=(i == FF // 128 - 1))
            for j in range(NC_D):
                nc.vector.tensor_add(h[j], h[j], dps[j])

        # ---- final rmsnorm ----
        ss = ptile([1, T], name="fss")
        for i in range(NC_D):
            sq = p_sq.tile([128, T], F32, tag="sq", name="fsq")
            nc.scalar.square(sq, h[i])
            nc.tensor.matmul(ss, ones128, sq, start=(i == 0),
                             stop=(i == NC_D - 1))
        srt = p_sm.tile([1, T], F32, tag="sm", name="fsrt")
        nc.scalar.activation(srt, ss, AF.Sqrt, bias=eps2[0:1], scale=1.0 / D)
        rs = p_sm.tile([1, T], F32, tag="sm", name="frs")
        nc.vector.reciprocal(rs, srt)
        bc = ptile([128, T], name="fbc")
        nc.tensor.matmul(bc, ones1x, rs, start=True, stop=True)
        for i in range(NC_D):
            o = p_osb.tile([128, T], F32, tag="osb", name="osb")
            nc.vector.tensor_mul(o, h[i], bc)
            nc.vector.tensor_scalar_mul(o, o, nw[:, i:i + 1])
            nc.sync.dma_start(out=OUT[i * 128:(i + 1) * 128, :], in_=o)

        for p in reversed((cst, st, p_nrm, p_sq, p_sm, p_bcs, p_qn, p_t12,
                           p_qf, p_kloc, p_kbig, p_vsb, p_vtok, p_es, p_prod,
                           p_osb, p_wqkv, p_wo, p_wg, p_wu, p_wdn, psum, dram)):
            p.release()

    nc.compile()
    return nc


def _bf16(a):
    return np.asarray(a, np.float32).astype(ml_dtypes.bfloat16)


def _host_consts():
    """Per-core-independent constant arrays."""
    c = {}
    # rope permutation lhsT: out = ropeP.T @ x = rotate_half(x), per 64-block
    P = np.zeros((128, 128), np.float32)
    for blk in range(2):
        o = blk * 64
        for d_ in range(32):
            P[o + d_ + 32, o + d_] = -1.0
        for d_ in range(32, 64):
            P[o + d_ - 32, o + d_] = 1.0
    c["ropeP"] = _bf16(P)
    bs = np.zeros((128, 2), np.float32)
    bs[0:64, 0] = 1.0
    bs[64:128, 1] = 1.0
    c["blksum"] = bs
    c["ones128"] = np.ones((128, 1), np.float32)
    c["ones1x"] = np.ones((1, 128), np.float32)
    c["id128"] = _bf16(np.eye(128, dtype=np.float32))
    return c


def _host_masks(off):
    """Sliding-window masks. sm0/sm1/sm2 are position-uniform; smel/smer
    carry the sequence-boundary validity (differ per core)."""
    m0 = np.zeros((64, 128), np.float32)    # k tokens [128ci-64, 128ci)
    m1 = np.zeros((128, 128), np.float32)   # k tokens [128ci, 128ci+128)
    m2 = np.zeros((32, 128), np.float32)    # k tokens [128ci+128, +32)
    mel = np.zeros((12, 128), np.float32)   # k tokens [-12, 0) (ci=0)
    mer = np.zeros((12, 128), np.float32)   # k tokens [512, 524) (ci=3)
    for cq in range(128):
        for r in range(64):
            if abs((r - 64) - cq) <= WIN:
                m0[r, cq] = 1.0
        for r in range(32):
            if abs((r + 128) - cq) <= WIN:
                m2[r, cq] = 1.0
        for r in range(128):
            if abs(r - cq) <= WIN:
                m1[r, cq] = 1.0
        for r in range(12):
            lk = r - 12            # MEL pairs with ci=0: lq = cq
            if 0 <= off + lk < S and abs(lk - cq) <= WIN:
                mel[r, cq] = 1.0
            lk = T + r             # MER pairs with ci=3: lq = 384 + cq
            if 0 <= off + lk < S and abs(lk - (384 + cq)) <= WIN:
                mer[r, cq] = 1.0
    return {"sm0": _bf16(m0), "sm1": _bf16(m1), "sm2": _bf16(m2),
            "smel": _bf16(mel), "smer": _bf16(mer)}


def _host_rope(off):
    inv = 1.0 / (THETA ** (np.arange(0, HD, 2, dtype=np.float32) / HD))
    pos = np.arange(off, off + T, dtype=np.float32)
    ang = pos[:, None] * inv[None, :]          # [T, 32]
    emb = np.concatenate([ang, ang], axis=1)   # [T, 64]
    cosb = np.tile(np.cos(emb).T, (2, 1)).astype(np.float32)  # [128, T]
    sinb = np.tile(np.sin(emb).T, (2, 1)).astype(np.float32)
    return cosb, sinb


_CACHE = {}


def _prep_in_maps(ins):
    return _prep(**{k: ins[k] for k in (
        "inputs_embeds", "wq", "wk", "wv", "wo", "q_norm_w", "k_norm_w",
        "ln1_w", "ln2_w", "w_gate", "w_up", "w_down", "norm_w")})


def _prep(inputs_embeds, wq, wk, wv, wo, q_norm_w, k_norm_w, ln1_w, ln2_w,
          w_gate, w_up, w_down, norm_w):
    ln1 = np.asarray(ln1_w, np.float32)
    ln2 = np.asarray(ln2_w, np.float32)
    qcp = np.concatenate([np.arange(64) + QPERM[p][c] * 64
                          for c in range(8) for p in range(2)])
    wq_p = np.asarray(wq, np.float32)[:, :, qcp]
    wqkv = np.concatenate([wq_p,
                           np.asarray(wk, np.float32),
                           np.asarray(wv, np.float32)], axis=2)
    wqkv = _bf16(ln1[:, :, None] * wqkv)
    wgu = np.concatenate([np.asarray(w_gate, np.float32),
                          np.asarray(w_up, np.float32)], axis=2)
    wgu = _bf16(ln2[:, :, None] * wgu)
    wo_b = _bf16(np.asarray(wo, np.float32)[:, qcp, :])
    wdn_b = _bf16(w_down)

    qnw = np.asarray(q_norm_w, np.float32)   # [L, 64]
    knw = np.asarray(k_norm_w, np.float32)
    qbc = np.zeros((L, 2, 128), np.float32)
    kbc = np.zeros((L, 2, 128), np.float32)
    for li in range(L):
        for g in range(2):
            qbc[li, g, g * 64:(g + 1) * 64] = qnw[li] / np.sqrt(HD)
            kbc[li, g, g * 64:(g + 1) * 64] = knw[li]
    nwc = np.asarray(norm_w, np.float32).reshape(NC_D, 128).T.copy()  # [128,8]

    consts = _host_consts()
    x = np.asarray(inputs_embeds, np.float32)

    in_maps = []
    for c in range(NCORES):
        b, half = c // 2, c % 2
        off = half * T
        cosb, sinb = _host_rope(off)
        in_maps.append({
            "x": np.ascontiguousarray(x[b, off:off + T, :].T),
            "wqkv": wqkv, "wo": wo_b, "wgu": wgu, "wdn": wdn_b,
            "qbc": qbc, "kbc": kbc,
            "cosb": cosb, "sinb": sinb,
            "nw": nwc,
            **_host_masks(off),
            **consts,
        })
    return in_maps


def kernel(inputs_embeds, wq, wk, wv, wo, q_norm_w, k_norm_w, ln1_w, ln2_w,
           w_gate, w_up, w_down, norm_w, attention_mask):
    if "nc" not in _CACHE:
        _CACHE["nc"] = _build_program(NCORES)
    nc = _CACHE["nc"]
    in_maps = _prep(inputs_embeds, wq, wk, wv, wo, q_norm_w, k_norm_w, ln1_w,
                    ln2_w, w_gate, w_up, w_down, norm_w)
    res = bass_utils.run_bass_kernel_spmd(nc, in_maps,
                                          core_ids=list(range(NCORES)),
                                          trace=False)
    out = np.empty((B, S, D), np.float32)
    for c in range(NCORES):
        b, half = c // 2, c % 2
        off = half * T
        out[b, off:off + T, :] = res.results[c]["out"].T
    return out


if __name__ == "__main__":
    import reference
    ins = reference.setup_inputs()
    ins = {k: np.asarray(v) for k, v in ins.items()}
    got = kernel(**ins)
    print("out shape", got.shape)


def _make_runner(nc, in_maps):
    """Persistent jitted shard_map runner for timing (mirrors
    bass2jax.run_bass_via_pjrt but keeps the callable + device-resident
    inputs so repeated dispatches measure device time, not H2D)."""
    import jax
    from jax.sharding import Mesh, PartitionSpec, NamedSharding
    from jax.experimental.shard_map import shard_map
    from concourse import bass2jax

    bass2jax.install_neuronx_cc_hook()
    n_cores = len(in_maps)
    partition_name = (nc.partition_id_tensor.name
                      if nc.partition_id_tensor else None)
    in_names, out_names, out_avals, zero_outs = [], [], [], []
    for alloc in nc.m.functions[0].allocations:
        if not isinstance(alloc, mybir.MemoryLocationSet):
            continue
        name = alloc.memorylocations[0].name
        if alloc.kind == "ExternalInput":
            if name != partition_name:
                in_names.append(name)
        elif alloc.kind == "ExternalOutput":
            shape = tuple(alloc.tensor_shape)
            dtype = mybir.dt.np(alloc.dtype)
            out_names.append(name)
            out_avals.append(jax.core.ShapedArray(shape, dtype))
            zero_outs.append(np.zeros(shape, dtype))
    n_params = len(in_names)
    all_in_names = in_names + out_names
    if partition_name is not None:
        all_in_names.append(partition_name)
    donate = tuple(range(n_params, n_params + len(out_names)))

    def _body(*args):
        operands = list(args)
        if partition_name is not None:
            operands.append(bass2jax.partition_id_tensor())
        outs = bass2jax._bass_exec_p.bind(
            *operands,
            out_avals=tuple(out_avals),
            in_names=tuple(all_in_names),
            out_names=tuple(out_names),
            lowering_input_output_aliases=(),
            sim_require_finite=True,
            sim_require_nnan=True,
            nc=nc,
        )
        return tuple(outs)

    devices = jax.devices()[:n_cores]
    mesh = Mesh(np.asarray(devices), ("core",))
    n_outs = len(out_names)
    in_specs = (PartitionSpec("core"),) * (n_params + n_outs)
    out_specs = (PartitionSpec("core"),) * n_outs
    fn = jax.jit(
        shard_map(_body, mesh=mesh, in_specs=in_specs, out_specs=out_specs,
                  check_rep=False),
        donate_argnums=donate, keep_unused=True)
    sh = NamedSharding(mesh, PartitionSpec("core"))
    concat_in = [
        jax.device_put(
            np.concatenate([np.asarray(in_maps[c][n]) for c in range(n_cores)],
                           axis=0), sh)
        for n in in_names
    ]
    concat_zeros = [np.zeros((n_cores * z.shape[0], *z.shape[1:]), z.dtype)
                    for z in zero_outs]

    def run():
        zs = [jax.device_put(z, sh) for z in concat_zeros]
        outs = fn(*concat_in, *zs)
        jax.block_until_ready(outs)
        return outs

    return run, out_names, out_avals


def time_kernel(ins, iters=8):
    """Median-of-min wall time per dispatch, ns (includes dispatch overhead)."""
    import time as _t
    if "nc" not in _CACHE:
        _CACHE["nc"] = _build_program(NCORES)
    in_maps = _prep_in_maps(ins)
    run, _, _ = _make_runner(_CACHE["nc"], in_maps)
    run()  # compile + warm
    times = []
    for _ in range(iters):
        t0 = _t.perf_counter()
        run()
        times.append((_t.perf_counter() - t0) * 1e9)
    times.sort()
    print("dispatch times (us):", [f"{t/1e3:.0f}" for t in times])
    return times[0]


def _build_empty(n_cores=NCORES):
    """Minimal program with same-sized output — measures dispatch floor."""
    nc = bacc.Bacc("TRN2", target_bir_lowering=False, debug=False,
                   num_devices=n_cores)
    X = nc.dram_tensor("x", [D, T], F32, kind="ExternalInput").ap()
    OUT = nc.dram_tensor("out", [D, T], F32, kind="ExternalOutput").ap()
    with tile.TileContext(nc) as tc:
        with tc.tile_pool(name="sb", bufs=2) as sb:
            for i in range(NC_D):
                t_ = sb.tile([128, T], F32, tag="t", name="t")
                nc.sync.dma_start(out=t_, in_=X[i * 128:(i + 1) * 128, :])
                nc.sync.dma_start(out=OUT[i * 128:(i + 1) * 128, :], in_=t_)
    nc.compile()
    return nc


def time_empty(ins, iters=8):
    import time as _t
    nc = _build_empty(NCORES)
    maps = _prep_in_maps(ins)
    in_maps = [{"x": m["x"]} for m in maps]
    run, _, _ = _make_runner(nc, in_maps)
    run()
    times = []
    for _ in range(iters):
        t0 = _t.perf_counter()
        run()
        times.append((_t.perf_counter() - t0) * 1e9)
    times.sort()
    print("empty dispatch times (us):", [f"{t/1e3:.0f}" for t in times])
    return times[0]



# revision 11
# speedup vs baseline: 1.8815x; 1.8815x over previous
# Trainium2 Bass kernel for nn_Krop_81544249082422 (4-layer Qwen3-style
# transformer, alternating full / sliding-window attention).
#
# Sharding: 8 cores = (batch 4) x (seq-half 2). Each core owns 512 tokens of
# one batch element, feature-major ([feature, token]) through the whole stack.
# Cross-core traffic: pairwise K/V AllGather per full-attn layer; 12-token
# halo exchange per sliding layer.
import sys

for p in ("/opt/trn_rl_repo", "/opt/pypackages"):
    if p not in sys.path:
        sys.path.insert(0, p)

import numpy as np
import ml_dtypes

import concourse.bass as bass
import concourse.bacc as bacc
import concourse.mybir as mybir
import concourse.tile as tile
from concourse import bass_utils

F32 = mybir.dt.float32
BF16 = mybir.dt.bfloat16
AF = mybir.ActivationFunctionType

L, D, H, HK, HD, FF = 4, 1024, 16, 8, 64, 3072
WIN = 12
THETA = 1e6
EPS = 1e-6
B, S = 4, 1024
NCORES = 8
T = 512            # tokens per core
NC_D = D // 128    # 8 feature chunks
NC_T = T // 128    # 4 local token chunks
QKV_OUT = H * HD + 2 * HK * HD   # 2048
VAUG = HK * (HD + 1)             # 520: 8 kv heads x (64 + ones col)
KOFF = 32                        # ext-k column offset (local token 0 -> col 32)
KEXT = T + 2 * KOFF              # 576
KE = NC_T * 128 * 24             # 12288 (k edge block in halo exchange)
VE = 24 * VAUG                   # 12480
HALO = KE + VE                   # 24768
PAIRS = [[0, 1], [2, 3], [4, 5], [6, 7]]
# q-head slot layout: chunk c rows [0:64) = QPERM[0][c], rows [64:128) = QPERM[1][c].
# Chosen so each q head's GQA kv head sits at the same partition parity
# (matmul requires equal base partitions for lhsT and rhs).
QPERM = [[0, 1, 4, 5, 8, 9, 12, 13], [2, 3, 6, 7, 10, 11, 14, 15]]


def _build_program(n_cores=NCORES, cc=True):
    nc = bacc.Bacc("TRN2", target_bir_lowering=False, debug=False,
                   num_devices=n_cores)

    def collective(kind, op, replica_groups, ins, outs):
        if cc:
            nc.gpsimd.collective_compute(kind, op,
                                         replica_groups=replica_groups,
                                         ins=ins, outs=outs)
        else:
            # Timing stand-in for TimelineSim (single-core, no collectives):
            # copy own contribution into both halves of the output.
            assert kind == "AllGather"
            src, dst = ins[0], outs[0]
            half = dst.shape[0] // 2
            nc.sync.dma_start(out=dst[0:half], in_=src)
            nc.sync.dma_start(out=dst[half:2 * half], in_=src)

    def din(name, shape, dt=BF16):
        return nc.dram_tensor(name, shape, dt, kind="ExternalInput").ap()

    X = din("x", [D, T], F32)
    WQKV = din("wqkv", [L, D, QKV_OUT])
    WO = din("wo", [L, D, D])
    WGU = din("wgu", [L, D, 2 * FF])
    WDN = din("wdn", [L, FF, D])
    QBC = din("qbc", [L, 2, 128], F32)
    KBC = din("kbc", [L, 2, 128], F32)
    COSB = din("cosb", [128, T], F32)
    SINB = din("sinb", [128, T], F32)
    ROPEP = din("ropeP", [128, 128])
    BLKSUM = din("blksum", [128, 2], F32)
    ONES128 = din("ones128", [128, 1], F32)
    ONES1X = din("ones1x", [1, 128], F32)
    ID128 = din("id128", [128, 128])
    SM0 = din("sm0", [64, 128])
    SM1 = din("sm1", [128, 128])
    SM2 = din("sm2", [32, 128])
    SMEL = din("smel", [12, 128])
    SMER = din("smer", [12, 128])
    NW = din("nw", [128, NC_D], F32)
    OUT = nc.dram_tensor("out", [D, T], F32, kind="ExternalOutput").ap()

    with tile.TileContext(nc) as tc:
        cst = tc.alloc_tile_pool(name="cst", bufs=1)
        st = tc.alloc_tile_pool(name="st", bufs=1)
        p_nrm = tc.alloc_tile_pool(name="p_nrm", bufs=8)
        p_sq = tc.alloc_tile_pool(name="p_sq", bufs=2)
        p_sm = tc.alloc_tile_pool(name="p_sm", bufs=4)
        p_bcs = tc.alloc_tile_pool(name="p_bcs", bufs=2)
        p_qn = tc.alloc_tile_pool(name="p_qn", bufs=2)
        p_t12 = tc.alloc_tile_pool(name="p_t12", bufs=3)
        p_qf = tc.alloc_tile_pool(name="p_qf", bufs=8)
        p_kloc = tc.alloc_tile_pool(name="p_kloc", bufs=4)
        p_kbig = tc.alloc_tile_pool(name="p_kbig", bufs=4)
        p_vsb = tc.alloc_tile_pool(name="p_vsb", bufs=2)
        p_vtok = tc.alloc_tile_pool(name="p_vtok", bufs=8)
        p_es = tc.alloc_tile_pool(name="p_es", bufs=3)
        p_prod = tc.alloc_tile_pool(name="p_prod", bufs=24)
        p_osb = tc.alloc_tile_pool(name="p_osb", bufs=2)
        p_wqkv = tc.alloc_tile_pool(name="p_wqkv", bufs=8)
        p_wo = tc.alloc_tile_pool(name="p_wo", bufs=8)
        p_wg = tc.alloc_tile_pool(name="p_wg", bufs=9)
        p_wu = tc.alloc_tile_pool(name="p_wu", bufs=9)
        p_wdn = tc.alloc_tile_pool(name="p_wdn", bufs=3)
        psum = tc.alloc_tile_pool(name="psum", bufs=8, space="PSUM")
        dram = tc.alloc_tile_pool(name="dram", bufs=2, space="DRAM")

        def ptile(shape, dt=F32, name="ps"):
            return psum.tile(shape, dt, tag="acc", name=name)

        # ---- load constants ----
        cosb = cst.tile([128, T], F32, name="cosb")
        sinb = cst.tile([128, T], F32, name="sinb")
        ropeP = cst.tile([128, 128], BF16, name="ropeP")
        blksum = cst.tile([128, 2], F32, name="blksum")
        ones128 = cst.tile([128, 1], F32, name="ones128")
        ones1x = cst.tile([1, 128], F32, name="ones1x")
        id128 = cst.tile([128, 128], BF16, name="id128")
        sm0 = cst.tile([128, 128], BF16, name="sm0")
        sm1 = cst.tile([128, 128], BF16, name="sm1")
        sm2 = cst.tile([32, 128], BF16, name="sm2")
        smel = cst.tile([12, 128], BF16, name="smel")
        smer = cst.tile([12, 128], BF16, name="smer")
        nw = cst.tile([128, NC_D], F32, name="nw")
        qbc = cst.tile([2, L * 128], F32, name="qbc")
        kbc = cst.tile([2, L * 128], F32, name="kbc")
        eps2 = cst.tile([2, 1], F32, name="eps2")
        for t_, s_ in ((cosb, COSB), (sinb, SINB), (ropeP, ROPEP),
                       (blksum, BLKSUM), (ones128, ONES128), (ones1x, ONES1X),
                       (id128, ID128), (sm1, SM1), (sm2, SM2),
                       (smel, SMEL), (smer, SMER), (nw, NW)):
            nc.sync.dma_start(out=t_, in_=s_)
        nc.sync.dma_start(out=sm0[64:128, :], in_=SM0)
        for li in range(L):
            nc.sync.dma_start(out=qbc[:, li * 128:(li + 1) * 128], in_=QBC[li])
            nc.sync.dma_start(out=kbc[:, li * 128:(li + 1) * 128], in_=KBC[li])
        nc.vector.memset(eps2, EPS)

        # ---- residual stream ----
        h = []
        for i in range(NC_D):
            hi = st.tile([128, T], F32, name=f"h{i}")
            nc.sync.dma_start(out=hi, in_=X[i * 128:(i + 1) * 128, :])
            h.append(hi)

        def rmsnorm_to(inv_n, out_dt, wmul=None):
            """Compute per-token rstd of h and return list of normed tiles."""
            ss = ptile([1, T], name="ss")
            for i in range(NC_D):
                sq = p_sq.tile([128, T], F32, tag="sq", name="sq")
                nc.scalar.square(sq, h[i])
                nc.tensor.matmul(ss, ones128, sq, start=(i == 0),
                                 stop=(i == NC_D - 1))
            srt = p_sm.tile([1, T], F32, tag="sm", name="srt")
            nc.scalar.activation(srt, ss, AF.Sqrt, bias=eps2[0:1], scale=inv_n)
            rs = p_sm.tile([1, T], F32, tag="sm", name="rs")
            nc.vector.reciprocal(rs, srt)
            bc = ptile([128, T], name="bc")
            nc.tensor.matmul(bc, ones1x, rs, start=True, stop=True)
            outs = []
            for i in range(NC_D):
                o = p_nrm.tile([128, T], out_dt, tag="nrm", name=f"n{i}")
                nc.vector.tensor_mul(o, h[i], bc)
                if wmul is not None:
                    nc.vector.tensor_scalar_mul(o, o, wmul[:, i:i + 1])
                outs.append(o)
            return outs

        def qk_path(li, ps, bcw, out_ap):
            """q/k head-rmsnorm + rope on one [128,T] psum chunk -> out_ap(bf16)."""
            sq = p_sq.tile([128, T], F32, tag="sq", name="qsq")
            nc.scalar.square(sq, ps)
            ss2 = ptile([2, T], name="ss2")
            nc.tensor.matmul(ss2, blksum, sq, start=True, stop=True)
            srt2 = p_sm.tile([2, T], F32, tag="sm", name="srt2")
            nc.scalar.activation(srt2, ss2, AF.Sqrt, bias=eps2, scale=1.0 / HD)
            rs2 = p_sm.tile([2, T], F32, tag="sm", name="rs2")
            nc.vector.reciprocal(rs2, srt2)
            bcq = ptile([128, T], name="bcq")
            nc.tensor.matmul(bcq, bcw[:, li * 128:(li + 1) * 128], rs2,
                             start=True, stop=True)
            bcs = p_bcs.tile([128, T], F32, tag="bcs", name="bcs")
            nc.vector.tensor_copy(bcs, bcq)
            qn = p_qn.tile([128, T], BF16, tag="qn", name="qn")
            nc.vector.tensor_mul(qn, ps, bcs)
            pp = ptile([128, T], name="pp")
            nc.tensor.matmul(pp, ropeP, qn, start=True, stop=True)
            t1 = p_t12.tile([128, T], F32, tag="t12", name="t1")
            nc.vector.tensor_mul(t1, qn, cosb)
            t2 = p_t12.tile([128, T], F32, tag="t12", name="t2")
            nc.vector.tensor_mul(t2, pp, sinb)
            nc.vector.tensor_add(out_ap, t1, t2)

        for li in range(L):
            sliding = (li % 2 == 1)
            n = rmsnorm_to(1.0 / D, BF16)

            # ---- QKV projection + q/k norm/rope + v transpose ----
            wq_sb = []
            for i in range(NC_D):
                w = p_wqkv.tile([128, QKV_OUT], BF16, tag="wqkv", name="wqkv_sb")
                nc.sync.dma_start(out=w, in_=WQKV[li, i * 128:(i + 1) * 128, :])
                wq_sb.append(w)

            qf = []
            kdst = []   # full: kloc tiles [128,T]; sliding: ext_k tiles [128,KEXT]
            if sliding:
                for c in range(NC_T):
                    ek = p_kbig.tile([128, KEXT], BF16, tag="kbig", name=f"extk{c}")
                    kdst.append(ek)
            vdst = []   # local token-major v (+ones cols): [4][128, VAUG]
            for tci in range(NC_T):
                vt = p_vtok.tile([128, VAUG], BF16, tag="vtok", name=f"vt{tci}")
                nc.vector.memset(vt, 1.0)
                vdst.append(vt)
            vf12 = vl12 = vhL = vhR = None
            if sliding:
                vf12 = p_vsb.tile([12, VAUG], BF16, tag="vedge", bufs=8,
                                  name="vf12")
                vl12 = p_vsb.tile([12, VAUG], BF16, tag="vedge", bufs=8,
                                  name="vl12")
                vhL = p_vsb.tile([12, VAUG], BF16, tag="vedge", bufs=8,
                                 name="vhL")
                vhR = p_vsb.tile([12, VAUG], BF16, tag="vedge", bufs=8,
                                 name="vhR")
                for t_ in (vf12, vl12, vhL, vhR):
                    nc.vector.memset(t_, 1.0)

            for j in range(QKV_OUT // 128):
                ps = ptile([128, T], name="qkv_ps")
                for i in range(NC_D):
                    nc.tensor.matmul(ps, wq_sb[i][:, j * 128:(j + 1) * 128],
                                     n[i], start=(i == 0), stop=(i == NC_D - 1))
                if j < 8:
                    q = p_qf.tile([128, T], BF16, tag="qf", name=f"qf{j}")
                    qk_path(li, ps, qbc, q)
                    qf.append(q)
                elif j < 12:
                    c = j - 8
                    if sliding:
                        qk_path(li, ps, kbc, kdst[c][:, KOFF:KOFF + T])
                    else:
                        kl = p_kloc.tile([128, T], BF16, tag="kloc",
                                         name=f"kloc{c}")
                        qk_path(li, ps, kbc, kl)
                        kdst.append(kl)
                else:
                    c = j - 12
                    vsb = p_vsb.tile([128, T], BF16, tag="vsb", name="vsb")
                    nc.vector.tensor_copy(vsb, ps)
                    # token-aligned transposes -> v_tok[tc]
                    for tci in range(NC_T):
                        tr = psum.tile([128, 128], BF16, tag="acc", name="tr")
                        nc.tensor.transpose(
                            tr, vsb[:, tci * 128:(tci + 1) * 128], id128)
                        for hh in range(2):
                            kv = 2 * c + hh
                            nc.vector.tensor_copy(
                                vdst[tci][:, kv * (HD + 1):kv * (HD + 1) + HD],
                                tr[:, hh * HD:(hh + 1) * HD])
                    if sliding:
                        # edge staging: own first/last 12 token rows of v
                        for (stage, a) in ((vf12, 0), (vl12, T - 12)):
                            tre = psum.tile([128, 128], BF16, tag="acc",
                                            name="tre")
                            nc.tensor.transpose(tre[0:12, :], vsb[:, a:a + 12],
                                                id128)
                            for hh in range(2):
                                kv = 2 * c + hh
                                nc.vector.tensor_copy(
                                    stage[:, kv * (HD + 1):kv * (HD + 1) + HD],
                                    tre[0:12, hh * HD:(hh + 1) * HD])

            # ---- K/V exchange ----
            if not sliding:
                cc_in = dram.tile([NC_T, 128, T + VAUG], BF16, tag="ccin",
                                  name="cc_in")
                cc_out = dram.tile([2 * NC_T, 128, T + VAUG], BF16, tag="ccout",
                                   name="cc_out")
                for c in range(NC_T):
                    nc.sync.dma_start(out=cc_in[c, :, 0:T], in_=kdst[c])
                    nc.sync.dma_start(out=cc_in[c, :, T:T + VAUG], in_=vdst[c])
                collective(
                    "AllGather", mybir.AluOpType.bypass, replica_groups=PAIRS,
                    ins=[cc_in.opt()], outs=[cc_out.opt()])
                k_full = []
                for i in range(NC_T):
                    kfl = p_kbig.tile([128, S], BF16, tag="kbig", name=f"kfull{i}")
                    nc.sync.dma_start(out=kfl[:, 0:T], in_=cc_out[i, :, 0:T])
                    nc.sync.dma_start(out=kfl[:, T:S],
                                      in_=cc_out[NC_T + i, :, 0:T])
                    k_full.append(kfl)
                v_aug = []
                for tci in range(2 * NC_T):
                    va = p_vtok.tile([128, VAUG], BF16, tag="vtok",
                                     name=f"vaug{tci}")
                    nc.sync.dma_start(out=va, in_=cc_out[tci, :, T:T + VAUG])
                    v_aug.append(va)
            else:
                cc_in = dram.tile([HALO], BF16, tag="ccin", name="cc_in_s")
                cc_out = dram.tile([2 * HALO], BF16, tag="ccout",
                                   name="cc_out_s")
                kv_view = cc_in[0:KE].rearrange("(c p w) -> c p w", c=NC_T, p=128)
                vv_view = cc_in[KE:HALO].rearrange("(p f) -> p f", p=24)
                for c in range(NC_T):
                    nc.sync.dma_start(out=kv_view[c, :, 0:12],
                                      in_=kdst[c][:, KOFF:KOFF + 12])
                    nc.sync.dma_start(out=kv_view[c, :, 12:24],
                                      in_=kdst[c][:, KOFF + T - 12:KOFF + T])
                nc.sync.dma_start(out=vv_view[0:12, :], in_=vf12)
                nc.sync.dma_start(out=vv_view[12:24, :], in_=vl12)
                collective(
                    "AllGather", mybir.AluOpType.bypass, replica_groups=PAIRS,
                    ins=[cc_in.opt()], outs=[cc_out.opt()])
                e0k = cc_out[0:KE].rearrange("(c p w) -> c p w", c=NC_T, p=128)
                e1k = cc_out[HALO:HALO + KE].rearrange("(c p w) -> c p w",
                                                       c=NC_T, p=128)
                e0v = cc_out[KE:HALO].rearrange("(p f) -> p f", p=24)
                e1v = cc_out[HALO + KE:2 * HALO].rearrange("(p f) -> p f", p=24)
                for c in range(NC_T):
                    nc.sync.dma_start(out=kdst[c][:, KOFF - 12:KOFF],
                                      in_=e0k[c, :, 12:24])
                    nc.sync.dma_start(out=kdst[c][:, KOFF + T:KOFF + T + 12],
                                      in_=e1k[c, :, 0:12])
                nc.sync.dma_start(out=vhL, in_=e0v[12:24, :])
                nc.sync.dma_start(out=vhR, in_=e1v[0:12, :])

            # ---- attention ----
            ao = []
            for i in range(NC_D):
                a = p_nrm.tile([128, T], BF16, tag="nrm", name=f"ao{i}")
                ao.append(a)
            for sl in range(H):
                qc, p = sl // 2, sl % 2
                kv = QPERM[p][qc] // 2
                fc, ro = qc // 2, p * HD
                qo = p * HD
                if not sliding:
                    ctx = ptile([HD + 1, T], name="ctx")
                    for kc in range(S // 128):
                        sT = ptile([128, T], name="sT")
                        nc.tensor.matmul(
                            sT, k_full[fc][ro:ro + HD, kc * 128:(kc + 1) * 128],
                            qf[qc][qo:qo + HD, :], start=True, stop=True)
                        es = p_es.tile([128, T], BF16, tag="es", name="es")
                        nc.scalar.activation(es, sT, AF.Exp)
                        nc.tensor.matmul(
                            ctx, v_aug[kc][:, kv * (HD + 1):(kv + 1) * (HD + 1)],
                            es, start=(kc == 0), stop=(kc == S // 128 - 1))
                    dn = p_sm.tile([1, T], F32, tag="sm", name="dn")
                    nc.vector.reciprocal(dn, ctx[HD:HD + 1, :])
                    bcd = ptile([HD, T], name="bcd")
                    nc.tensor.matmul(bcd, ones1x[0:1, 0:HD], dn,
                                     start=True, stop=True)
                    bcds = p_bcs.tile([128, T], F32, tag="bcs", name="bcds")
                    nc.vector.tensor_copy(bcds[0:HD, :], bcd)
                    nc.vector.tensor_mul(ao[qc][qo:qo + HD, :], ctx[0:HD, :],
                                         bcds[0:HD, :])
                else:
                    vcol = slice(kv * (HD + 1), (kv + 1) * (HD + 1))
                    for ci in range(NC_T):
                        # window pieces (k cols in ext coords, 32-aligned v rows):
                        #  p0: tokens [128ci-32,128ci)   / edge [-12,0) at ci=0
                        #  p1: tokens [128ci, 128ci+128)
                        #  p2: tokens [128ci+128, +32)   / edge [512,524) at ci=3
                        qs = qf[qc][qo:qo + HD, ci * 128:(ci + 1) * 128]
                        pieces = []
                        if ci == 0:
                            pieces.append((12, 0, KOFF - 12,
                                           vhL[0:12, vcol], smel))
                        else:
                            pieces.append(
                                (64, 64, KOFF + ci * 128 - 64,
                                 vdst[ci - 1][64:128, vcol], sm0[64:128, :]))
                        pieces.append(
                            (128, 0, KOFF + ci * 128, vdst[ci][:, vcol], sm1))
                        if ci == NC_T - 1:
                            pieces.append((12, 0, KOFF + T,
                                           vhR[0:12, vcol], smer))
                        else:
                            pieces.append(
                                (32, 0, KOFF + (ci + 1) * 128,
                                 vdst[ci + 1][0:32, vcol], sm2))
                        ctx = ptile([HD + 1, 128], name="ctxs")
                        for pi, (rows, pb, kcol, vap, mask) in enumerate(pieces):
                            sp = ptile([pb + rows, 128], name="sp")
                            nc.tensor.matmul(
                                sp[pb:pb + rows, :],
                                kdst[fc][ro:ro + HD, kcol:kcol + rows],
                                qs, start=True, stop=True)
                            ep = p_es.tile([pb + rows, 128], BF16, tag="es",
                                           name="ep")
                            nc.scalar.activation(ep[pb:pb + rows, :],
                                                 sp[pb:pb + rows, :], AF.Exp)
                            nc.vector.tensor_mul(ep[pb:pb + rows, :],
                                                 ep[pb:pb + rows, :], mask)
                            nc.tensor.matmul(ctx, vap, ep[pb:pb + rows, :],
                                             start=(pi == 0),
                                             stop=(pi == len(pieces) - 1))
                        dn = p_sm.tile([1, 128], F32, tag="sm", name="dns")
                        nc.vector.reciprocal(dn, ctx[HD:HD + 1, :])
                        bcd = ptile([HD, 128], name="bcds_ps")
                        nc.tensor.matmul(bcd, ones1x[0:1, 0:HD], dn,
                                         start=True, stop=True)
                        bcds = p_bcs.tile([128, T], F32, tag="bcs", name="bcdss")
                        nc.vector.tensor_copy(bcds[0:HD, 0:128], bcd)
                        nc.vector.tensor_mul(
                            ao[qc][qo:qo + HD, ci * 128:(ci + 1) * 128],
                            ctx[0:HD, :], bcds[0:HD, 0:128])

            # ---- output projection + residual ----
            wo_sb = []
            for i in range(NC_D):
                w = p_wo.tile([128, D], BF16, tag="wo", name="wo_sb")
                nc.sync.dma_start(out=w, in_=WO[li, i * 128:(i + 1) * 128, :])
                wo_sb.append(w)
            for j in range(NC_D):
                ps = ptile([128, T], name="wo_ps")
                for i in range(NC_D):
                    nc.tensor.matmul(ps, wo_sb[i][:, j * 128:(j + 1) * 128],
                                     ao[i], start=(i == 0), stop=(i == NC_D - 1))
                nc.vector.tensor_add(h[j], h[j], ps)

            # ---- MLP ----
            n2 = rmsnorm_to(1.0 / D, BF16)
            prod = []
            for g in range(6):
                wg_sb, wu_sb = [], []
                for i in range(NC_D):
                    wg_ = p_wg.tile([128, T], BF16, tag="wg", name="wg_sb")
                    nc.sync.dma_start(
                        out=wg_,
                        in_=WGU[li, i * 128:(i + 1) * 128, g * T:(g + 1) * T])
                    wg_sb.append(wg_)
                    wu_ = p_wu.tile([128, T], BF16, tag="wu", name="wu_sb")
                    nc.sync.dma_start(
                        out=wu_,
                        in_=WGU[li, i * 128:(i + 1) * 128,
                                FF + g * T:FF + (g + 1) * T])
                    wu_sb.append(wu_)
                for fl in range(4):
                    gps = ptile([128, T], name="gps")
                    for i in range(NC_D):
                        nc.tensor.matmul(gps, wg_sb[i][:, fl * 128:(fl + 1) * 128],
                                         n2[i], start=(i == 0),
                                         stop=(i == NC_D - 1))
                    ups = ptile([128, T], name="ups")
                    for i in range(NC_D):
                        nc.tensor.matmul(ups, wu_sb[i][:, fl * 128:(fl + 1) * 128],
                                         n2[i], start=(i == 0),
                                         stop=(i == NC_D - 1))
                    gsb = p_sq.tile([128, T], BF16, tag="sq", name="gsb")
                    nc.scalar.activation(gsb, gps, AF.Silu)
                    pr = p_prod.tile([128, T], BF16, tag="prod",
                                     name=f"prod{g * 4 + fl}")
                    nc.vector.tensor_mul(pr, gsb, ups)
                    prod.append(pr)
            dps = [ptile([128, T], name=f"dps{j}") for j in range(NC_D)]
            for i in range(FF // 128):
                wd = p_wdn.tile([128, D], BF16, tag="wdn", name="wd_sb")
                nc.sync.dma_start(out=wd, in_=WDN[li, i * 128:(i + 1) * 128, :])
                for j in range(NC_D):
                    nc.tensor.matmul(dps[j], wd[:, j * 128:(j + 1) * 128],
                                     prod[i], start=(i == 0),
                                     stop=(i == FF // 128 - 1))
            for j in range(NC_D):
                nc.vector.tensor_add(h[j], h[j], dps[j])

        # ---- final rmsnorm ----
        ss = ptile([1, T], name="fss")
        for i in range(NC_D):
            sq = p_sq.tile([128, T], F32, tag="sq", name="fsq")
            nc.scalar.square(sq, h[i])
            nc.tensor.matmul(ss, ones128, sq, start=(i == 0),
                             stop=(i == NC_D - 1))
        srt = p_sm.tile([1, T], F32, tag="sm", name="fsrt")
        nc.scalar.activation(srt, ss, AF.Sqrt, bias=eps2[0:1], scale=1.0 / D)
        rs = p_sm.tile([1, T], F32, tag="sm", name="frs")
        nc.vector.reciprocal(rs, srt)
        bc = ptile([128, T], name="fbc")
        nc.tensor.matmul(bc, ones1x, rs, start=True, stop=True)
        for i in range(NC_D):
            o = p_osb.tile([128, T], F32, tag="osb", name="osb")
            nc.vector.tensor_mul(o, h[i], bc)
            nc.vector.tensor_scalar_mul(o, o, nw[:, i:i + 1])
            nc.sync.dma_start(out=OUT[i * 128:(i + 1) * 128, :], in_=o)

        for p in reversed((cst, st, p_nrm, p_sq, p_sm, p_bcs, p_qn, p_t12,
                           p_qf, p_kloc, p_kbig, p_vsb, p_vtok, p_es, p_prod,
                           p_osb, p_wqkv, p_wo, p_wg, p_wu, p_wdn, psum, dram)):
            p.release()

    nc.compile()
    return nc


def _bf16(a):
    return np.asarray(a, np.float32).astype(ml_dtypes.bfloat16)


def _host_consts():
    """Per-core-independent constant arrays."""
    c = {}
    # rope permutation lhsT: out = ropeP.T @ x = rotate_half(x), per 64-block
    P = np.zeros((128, 128), np.float32)
    for blk in range(2):
        o = blk * 64
        for d_ in range(32):
            P[o + d_ + 32, o + d_] = -1.0
        for d_ in range(32, 64):
            P[o + d_ - 32, o + d_] = 1.0
    c["ropeP"] = _bf16(P)
    bs = np.zeros((128, 2), np.float32)
    bs[0:64, 0] = 1.0
    bs[64:128, 1] = 1.0
    c["blksum"] = bs
    c["ones128"] = np.ones((128, 1), np.float32)
    c["ones1x"] = np.ones((1, 128), np.float32)
    c["id128"] = _bf16(np.eye(128, dtype=np.float32))
    return c


def _host_masks(off):
    """Sliding-window masks. sm0/sm1/sm2 are position-uniform; smel/smer
    carry the sequence-boundary validity (differ per core)."""
    m0 = np.zeros((64, 128), np.float32)    # k tokens [128ci-64, 128ci)
    m1 = np.zeros((128, 128), np.float32)   # k tokens [128ci, 128ci+128)
    m2 = np.zeros((32, 128), np.float32)    # k tokens [128ci+128, +32)
    mel = np.zeros((12, 128), np.float32)   # k tokens [-12, 0) (ci=0)
    mer = np.zeros((12, 128), np.float32)   # k tokens [512, 524) (ci=3)
    for cq in range(128):
        for r in range(64):
            if abs((r - 64) - cq) <= WIN:
                m0[r, cq] = 1.0
        for r in range(32):
            if abs((r + 128) - cq) <= WIN:
                m2[r, cq] = 1.0
        for r in range(128):
            if abs(r - cq) <= WIN:
                m1[r, cq] = 1.0
        for r in range(12):
            lk = r - 12            # MEL pairs with ci=0: lq = cq
            if 0 <= off + lk < S and abs(lk - cq) <= WIN:
                mel[r, cq] = 1.0
            lk = T + r             # MER pairs with ci=3: lq = 384 + cq
            if 0 <= off + lk < S and abs(lk - (384 + cq)) <= WIN:
                mer[r, cq] = 1.0
    return {"sm0": _bf16(m0), "sm1": _bf16(m1), "sm2": _bf16(m2),
            "smel": _bf16(mel), "smer": _bf16(mer)}


def _host_rope(off):
    inv = 1.0 / (THETA ** (np.arange(0, HD, 2, dtype=np.float32) / HD))
    pos = np.arange(off, off + T, dtype=np.float32)
    ang = pos[:, None] * inv[None, :]          # [T, 32]
    emb = np.concatenate([ang, ang], axis=1)   # [T, 64]
    cosb = np.tile(np.cos(emb).T, (2, 1)).astype(np.float32)  # [128, T]
    sinb = np.tile(np.sin(emb).T, (2, 1)).astype(np.float32)
    return cosb, sinb


_CACHE = {}


def _prep_in_maps(ins):
    return _prep(**{k: ins[k] for k in (
        "inputs_embeds", "wq", "wk", "wv", "wo", "q_norm_w", "k_norm_w",
        "ln1_w", "ln2_w", "w_gate", "w_up", "w_down", "norm_w")})


def _prep(inputs_embeds, wq, wk, wv, wo, q_norm_w, k_norm_w, ln1_w, ln2_w,
          w_gate, w_up, w_down, norm_w):
    ln1 = np.asarray(ln1_w, np.float32)
    ln2 = np.asarray(ln2_w, np.float32)
    qcp = np.concatenate([np.arange(64) + QPERM[p][c] * 64
                          for c in range(8) for p in range(2)])
    wq_p = np.asarray(wq, np.float32)[:, :, qcp]
    wqkv = np.concatenate([wq_p,
                           np.asarray(wk, np.float32),
                           np.asarray(wv, np.float32)], axis=2)
    wqkv = _bf16(ln1[:, :, None] * wqkv)
    wgu = np.concatenate([np.asarray(w_gate, np.float32),
                          np.asarray(w_up, np.float32)], axis=2)
    wgu = _bf16(ln2[:, :, None] * wgu)
    wo_b = _bf16(np.asarray(wo, np.float32)[:, qcp, :])
    wdn_b = _bf16(w_down)

    qnw = np.asarray(q_norm_w, np.float32)   # [L, 64]
    knw = np.asarray(k_norm_w, np.float32)
    qbc = np.zeros((L, 2, 128), np.float32)
    kbc = np.zeros((L, 2, 128), np.float32)
    for li in range(L):
        for g in range(2):
            qbc[li, g, g * 64:(g + 1) * 64] = qnw[li] / np.sqrt(HD)
            kbc[li, g, g * 64:(g + 1) * 64] = knw[li]
    nwc = np.asarray(norm_w, np.float32).reshape(NC_D, 128).T.copy()  # [128,8]

    consts = _host_consts()
    x = np.asarray(inputs_embeds, np.float32)

    in_maps = []
    for c in range(NCORES):
        b, half = c // 2, c % 2
        off = half * T
        cosb, sinb = _host_rope(off)
        in_maps.append({
            "x": np.ascontiguousarray(x[b, off:off + T, :].T),
            "wqkv": wqkv, "wo": wo_b, "wgu": wgu, "wdn": wdn_b,
            "qbc": qbc, "kbc": kbc,
            "cosb": cosb, "sinb": sinb,
            "nw": nwc,
            **_host_masks(off),
            **consts,
        })
    return in_maps


def kernel(inputs_embeds, wq, wk, wv, wo, q_norm_w, k_norm_w, ln1_w, ln2_w,
           w_gate, w_up, w_down, norm_w, attention_mask):
    if "nc" not in _CACHE:
        _CACHE["nc"] = _build_program(NCORES)
    nc = _CACHE["nc"]
    in_maps = _prep(inputs_embeds, wq, wk, wv, wo, q_norm_w, k_norm_w, ln1_w,
                    ln2_w, w_gate, w_up, w_down, norm_w)
    res = bass_utils.run_bass_kernel_spmd(nc, in_maps,
                                          core_ids=list(range(NCORES)),
                                          trace=False)
    out = np.empty((B, S, D), np.float32)
    for c in range(NCORES):
        b, half = c // 2, c % 2
        off = half * T
        out[b, off:off + T, :] = res.results[c]["out"].T
    return out


if __name__ == "__main__":
    import reference
    ins = reference.setup_inputs()
    ins = {k: np.asarray(v) for k, v in ins.items()}
    got = kernel(**ins)
    print("out shape", got.shape)


def _make_runner(nc, in_maps, reps=1):
    """Persistent jitted shard_map runner for timing (mirrors
    bass2jax.run_bass_via_pjrt but keeps the callable + device-resident
    inputs so repeated dispatches measure device time, not H2D).

    reps>1 chains the NEFF execution `reps` times inside one jit call
    (output buffers of exec i feed exec i+1), so per-exec HW time can be
    measured as a slope with dispatch overhead cancelled."""
    import jax
    from jax.sharding import Mesh, PartitionSpec, NamedSharding
    from jax.experimental.shard_map import shard_map
    from concourse import bass2jax

    bass2jax.install_neuronx_cc_hook()
    n_cores = len(in_maps)
    partition_name = (nc.partition_id_tensor.name
                      if nc.partition_id_tensor else None)
    in_names, out_names, out_avals, zero_outs = [], [], [], []
    for alloc in nc.m.functions[0].allocations:
        if not isinstance(alloc, mybir.MemoryLocationSet):
            continue
        name = alloc.memorylocations[0].name
        if alloc.kind == "ExternalInput":
            if name != partition_name:
                in_names.append(name)
        elif alloc.kind == "ExternalOutput":
            shape = tuple(alloc.tensor_shape)
            dtype = mybir.dt.np(alloc.dtype)
            out_names.append(name)
            out_avals.append(jax.core.ShapedArray(shape, dtype))
            zero_outs.append(np.zeros(shape, dtype))
    n_params = len(in_names)
    all_in_names = in_names + out_names
    if partition_name is not None:
        all_in_names.append(partition_name)
    donate = tuple(range(n_params, n_params + len(out_names)))

    def _body(*args):
        operands = list(args)
        if partition_name is not None:
            operands.append(bass2jax.partition_id_tensor())
        outs = bass2jax._bass_exec_p.bind(
            *operands,
            out_avals=tuple(out_avals),
            in_names=tuple(all_in_names),
            out_names=tuple(out_names),
            lowering_input_output_aliases=(),
            sim_require_finite=True,
            sim_require_nnan=True,
            nc=nc,
        )
        return tuple(outs)

    devices = jax.devices()[:n_cores]
    mesh = Mesh(np.asarray(devices), ("core",))
    n_outs = len(out_names)
    in_specs = (PartitionSpec("core"),) * (n_params + n_outs)
    out_specs = (PartitionSpec("core"),) * n_outs
    fn = jax.jit(
        shard_map(_body, mesh=mesh, in_specs=in_specs, out_specs=out_specs,
                  check_rep=False),
        donate_argnums=donate, keep_unused=True)
    sh = NamedSharding(mesh, PartitionSpec("core"))
    concat_in = [
        jax.device_put(
            np.concatenate([np.asarray(in_maps[c][n]) for c in range(n_cores)],
                           axis=0), sh)
        for n in in_names
    ]
    concat_zeros = [np.zeros((n_cores * z.shape[0], *z.shape[1:]), z.dtype)
                    for z in zero_outs]

    def run(k=1):
        """Chain k executions: outputs of exec i are the (donated) output
        buffers of exec i+1, so the device work serializes while the host
        dispatches asynchronously."""
        zs = [jax.device_put(z, sh) for z in concat_zeros]
        jax.block_until_ready(zs)
        import time as _t
        t0 = _t.perf_counter()
        for _ in range(k):
            zs = list(fn(*concat_in, *zs))
        jax.block_until_ready(zs)
        return _t.perf_counter() - t0, zs

    return run, out_names, out_avals


def time_kernel(ins, iters=6, r_lo=1, r_hi=17):
    """Per-execution HW time via chained-execution slope:
    (T(r_hi) - T(r_lo)) / (r_hi - r_lo). Dispatch/RPC overhead cancels."""
    if "nc" not in _CACHE:
        _CACHE["nc"] = _build_program(NCORES)
    in_maps = _prep_in_maps(ins)
    run, _, _ = _make_runner(_CACHE["nc"], in_maps)
    run()  # compile + warm

    def best(k):
        times = sorted(run(k)[0] * 1e9 for _ in range(iters))
        print(f"chain k={k} times (us):", [f"{t/1e3:.0f}" for t in times])
        return times[0]

    t_lo = best(r_lo)
    t_hi = best(r_hi)
    return (t_hi - t_lo) / (r_hi - r_lo)


def _build_empty(n_cores=NCORES):
    """Minimal program with same-sized output — measures dispatch floor."""
    nc = bacc.Bacc("TRN2", target_bir_lowering=False, debug=False,
                   num_devices=n_cores)
    X = nc.dram_tensor("x", [D, T], F32, kind="ExternalInput").ap()
    OUT = nc.dram_tensor("out", [D, T], F32, kind="ExternalOutput").ap()
    with tile.TileContext(nc) as tc:
        with tc.tile_pool(name="sb", bufs=2) as sb:
            for i in range(NC_D):
                t_ = sb.tile([128, T], F32, tag="t", name="t")
                nc.sync.dma_start(out=t_, in_=X[i * 128:(i + 1) * 128, :])
                nc.sync.dma_start(out=OUT[i * 128:(i + 1) * 128, :], in_=t_)
    nc.compile()
    return nc


def time_empty(ins, iters=8):
    import time as _t
    nc = _build_empty(NCORES)
    maps = _prep_in_maps(ins)
    in_maps = [{"x": m["x"]} for m in maps]
    run, _, _ = _make_runner(nc, in_maps)
    run()
    times = []
    for _ in range(iters):
        t0 = _t.perf_counter()
        run()
        times.append((_t.perf_counter() - t0) * 1e9)
    times.sort()
    print("empty dispatch times (us):", [f"{t/1e3:.0f}" for t in times])
    return times[0]

